# revision 1
# baseline (speedup 1.0000x reference)
"""Multi-head self-attention with RoPE — Trainium2 Bass kernel, 8 NeuronCores.

Sharding: core c = 2*b + g handles batch b = c//2 and head-group g = c%2
(8 of the 16 heads).  Within each batch pair the cores AllGather their
normalized attention outputs (O^T, bf16) and both run the full output
projection redundantly; the host keeps the even core's y.  No reduction
on the host.

Per-core dataflow (matmuls bf16, fp32 PSUM accumulation):
  xT [E, L] bf16 (pre-transposed on host)
  QKV:   Q^T/K^T pair tiles via W-stationary matmuls; V natural [L, 512].
  RoPE:  weights pre-permuted on host to de-interleave even/odd dims, so
         rotate-half becomes a 32-partition block swap (SBUF->SBUF DMA);
         cos/sin multiplies on GPSIMD, combine add on DVE.
  Scores:S^T half-tiles [Lk=128, Lq=512] per head, heads row-tiled on PE;
         three halves share a [128, 1536] PSUM tile (double buffered) so
         exp runs as few, wide ACT instructions overlapped with PE.
  Softmax: denominator via ones-column appended to V (PSUM partition 64
         of O^T); reciprocal_approx + gpsimd partition_broadcast.
  AV:    O^T[65, 512] += V_aug^T A^T over 16 Lk chunks.
  Proj:  y[lq] = Ocat^T.T @ w_out, fp32 [L, E].
"""

import contextlib
import functools

import numpy as np
import ml_dtypes

import concourse.bass as bass
import concourse.mybir as mybir
import concourse.tile as tile
from concourse import bacc
from concourse.bass_utils import run_bass_kernel_spmd

BF16 = mybir.dt.bfloat16
F32 = mybir.dt.float32
N_CORES = 8
ROPE_THETA = 10000.0

B_FULL, L_FULL, E_FULL = 4, 2048, 1024
H_FULL = 16


def _emit(tc, nc, xT, wqkv, wout, cosT, sinT, y, L, E, HC, D, taps=None, use_collective=True):
    P = 128
    EC = E // P                 # E chunks of 128 (contraction)
    NPAIR = HC // 2             # head pairs per core
    LT = L // 512               # 512-wide L tiles
    LKC = L // P                # 128-wide Lk chunks
    A = HC * D                  # local attention width (512)
    scale = 1.0 / float(np.sqrt(D))
    Exp = mybir.ActivationFunctionType.Exp

    ctx = contextlib.ExitStack()
    pool = ctx.enter_context(tc.tile_pool(name="sb", bufs=1))
    psum = ctx.enter_context(tc.tile_pool(name="ps", bufs=1, space="PSUM"))
    work = ctx.enter_context(tc.tile_pool(name="wk", bufs=1))
    dram = ctx.enter_context(tc.tile_pool(name="dr", bufs=1, space="DRAM"))

    # ---- persistent SBUF buffers ----
    xt_sb = pool.tile([P, EC, L], BF16, tag="xbuf")
    wqkv_sb = pool.tile([P, EC, 3 * A], BF16, tag="wqkv")
    wout_sb = pool.tile([P, EC, E], BF16, tag="wout")
    cos_sb = pool.tile([P, L], BF16, tag="costab")
    sin_sb = pool.tile([P, L], BF16, tag="sintab")
    qk_sb = pool.tile([P, 2, NPAIR, L], BF16, tag="qk")      # [pair-rows, q/k, pair, L]
    vaug_sb = pool.tile([P, LKC, HC, D + 1], BF16, tag="vaug")
    ot_sb = pool.tile([64, HC, L], BF16, tag="ot")           # normalized O^T per head

    nc.sync.dma_start(xt_sb[:], xT.ap().rearrange("(c p) l -> p c l", p=P))
    nc.sync.dma_start(wqkv_sb[:], wqkv.ap().rearrange("(c p) n -> p c n", p=P))
    nc.sync.dma_start(wout_sb[:], wout.ap().rearrange("(c p) n -> p c n", p=P))
    nc.sync.dma_start(cos_sb[:], cosT.ap())
    nc.sync.dma_start(sin_sb[:], sinT.ap())

    # ones column for the softmax denominator
    nc.vector.memset(vaug_sb[:, :, :, D : D + 1], 1.0)

    # ---- V = x @ Wv, natural [L, A] layout, 2 L-chunks per PSUM tile ----
    for vg in range(LKC // 2):
        ps = psum.tile([P, 1024], F32, tag="sc", bufs=2)
        for i in range(2):
            lt = vg * 2 + i
            for e in range(EC):
                nc.tensor.matmul(
                    ps[:, i * 512 : (i + 1) * 512],
                    lhsT=xt_sb[:, e, lt * P : (lt + 1) * P],
                    rhs=wqkv_sb[:, e, 2 * A : 3 * A],
                    start=(e == 0),
                    stop=(e == EC - 1),
                )
        nc.scalar.copy(
            out=vaug_sb[:, vg * 2 : (vg + 1) * 2, :, 0:D],
            in_=ps[:].rearrange("p (t h d) -> p t h d", h=HC, d=D),
        )

    # ---- Q^T / K^T + RoPE ----
    # psum tile cols: [q | k] for one 512-wide L tile
    for p in range(NPAIR):
        for lt in range(LT):
            ps = psum.tile([P, 1024], F32, tag="sc", bufs=2)
            for qk in range(2):
                wcol = qk * A + p * P
                for e in range(EC):
                    nc.tensor.matmul(
                        ps[:, qk * 512 : (qk + 1) * 512],
                        lhsT=wqkv_sb[:, e, wcol : wcol + P],
                        rhs=xt_sb[:, e, lt * 512 : (lt + 1) * 512],
                        start=(e == 0),
                        stop=(e == EC - 1),
                    )
            Lsl = slice(lt * 512, (lt + 1) * 512)
            tab = lambda sb: (
                sb[:, Lsl][:, None, :].to_broadcast([P, 2, 512])
            )
            qs = work.tile([P, 1024], BF16, tag="qs", bufs=3)
            nc.scalar.copy(out=qs[:], in_=ps[:])
            qs_v = qs[:].rearrange("p (q c) -> p q c", q=2)
            w = work.tile([P, 1024], BF16, tag="w", bufs=3)
            t = work.tile([P, 1024], BF16, tag="w", bufs=3)
            nc.gpsimd.tensor_mul(w[:].rearrange("p (q c) -> p q c", q=2), qs_v, tab(sin_sb))
            nc.gpsimd.tensor_mul(t[:].rearrange("p (q c) -> p q c", q=2), qs_v, tab(cos_sb))
            wsw = work.tile([P, 1024], BF16, tag="wsw", bufs=2)
            for blk in range(4):
                sb = blk ^ 1  # swap 32-row blocks pairwise
                nc.sync.dma_start(
                    wsw[blk * 32 : (blk + 1) * 32, :], w[sb * 32 : (sb + 1) * 32, :]
                )
            out_ap = qk_sb[:, :, p, Lsl]  # [P, 2, 512]
            nc.vector.tensor_add(
                out_ap,
                t[:].rearrange("p (q c) -> p q c", q=2),
                wsw[:].rearrange("p (q c) -> p q c", q=2),
            )

    # ---- attention + AllGather staging ----
    cc_half = NPAIR // 2 * P  # feature rows per collective (2 pairs x 128)
    cc_in = [
        dram.tile([cc_half, L], BF16, tag=f"ccin{i}", bufs=1, name=f"ccin{i}")
        for i in range(2)
    ]
    cc_out = [
        dram.tile([2, cc_half, L], BF16, tag=f"ccout{i}", bufs=1, name=f"ccout{i}")
        for i in range(2)
    ]

    for p in range(NPAIR):
        h0, h1 = 2 * p, 2 * p + 1
        for lq in range(LT):
            ot0 = psum.tile([65, 512], F32, tag="ot", bufs=2)
            ot1 = psum.tile([65, 512], F32, tag="ot", bufs=2)
            ots = (ot0, ot1)
            Lq = slice(lq * 512, (lq + 1) * 512)
            # halves: (head, lk) pairs in lk-major order, grouped 3 per
            # [128, 1536] psum tile so exp runs as wide ACT instructions.
            halves = [(hh, lk) for lk in range(LKC) for hh in range(2)]
            gi = 0
            while gi < len(halves):
                grp = halves[gi : gi + 3]
                nh = len(grp)
                ps = psum.tile([P, 1536], F32, tag="sc", bufs=2)
                for j, (hh, lk) in enumerate(grp):
                    nc.tensor.matmul(
                        ps[:, j * 512 : (j + 1) * 512],
                        lhsT=qk_sb[hh * 64 : (hh + 1) * 64, 1, p, lk * P : (lk + 1) * P],
                        rhs=qk_sb[hh * 64 : (hh + 1) * 64, 0, p, Lq],
                        start=True,
                        stop=True,
                    )
                at = work.tile([P, 1536], BF16, tag="at", bufs=4)
                nc.scalar.activation(at[:, : nh * 512], ps[:, : nh * 512], Exp, scale=scale)
                for j, (hh, lk) in enumerate(grp):
                    nc.tensor.matmul(
                        ots[hh][:],
                        lhsT=vaug_sb[:, lk, 2 * p + hh, :],
                        rhs=at[:, j * 512 : (j + 1) * 512],
                        start=(lk == 0),
                        stop=(lk == LKC - 1),
                    )
                gi += nh
            for hh, otp in ((0, ot0), (1, ot1)):
                # denominator: PSUM row 64 -> SBUF row 64 -> (DMA) row 0 ->
                # reciprocal -> broadcast to 64 partitions -> scale O^T.
                den = work.tile([65, 512], F32, tag="den", bufs=1)
                nc.vector.tensor_copy(out=den[64:65, :], in_=otp[64:65, :])
                den0 = work.tile([1, 512], F32, tag="den0", bufs=1)
                nc.sync.dma_start(den0[0:1, :], den[64:65, :])
                rec0 = work.tile([1, 512], F32, tag="rec0", bufs=1)
                nc.vector.reciprocal_approx_fast(rec0[0:1, :], den0[0:1, :])
                rbc = work.tile([64, 512], F32, tag="rbc", bufs=2)
                nc.gpsimd.partition_broadcast(rbc[:], rec0[0:1, :])
                nc.vector.tensor_mul(ot_sb[:, 2 * p + hh, Lq], otp[0:64, :], rbc[:])
        if p % 2 == 1:
            half = p // 2
            src = ot_sb[:, half * (NPAIR // 2) * 2 : (half + 1) * (NPAIR // 2) * 2, :]
            nc.sync.dma_start(
                cc_in[half][:].rearrange("(h d) l -> d h l", d=64),
                src,
            )
            if use_collective:
                nc.gpsimd.collective_compute(
                    "AllGather",
                    mybir.AluOpType.bypass,
                    replica_groups=[[2 * i, 2 * i + 1] for i in range(N_CORES // 2)],
                    ins=[cc_in[half][:].opt()],
                    outs=[cc_out[half][:].opt()],
                )
            else:  # timing-analysis build: stand-in DMAs, no collective
                nc.sync.dma_start(cc_out[half][0], cc_in[half][:])
                nc.sync.dma_start(cc_out[half][1], cc_in[half][:])

    # ---- gather Ocat^T into SBUF: [128, EC, L], global feature-major ----
    ocat_sb = pool.tile([P, EC, L], BF16, tag="xbuf")
    for g2 in range(2):
        for half in range(2):
            blk = cc_out[half][g2]  # [cc_half, L]
            for q in range(cc_half // P):
                f = g2 * (2 * cc_half) + half * cc_half + q * P  # global row
                nc.sync.dma_start(ocat_sb[:, f // P, :], blk[q * P : (q + 1) * P, :])

    if taps is not None:
        nc.sync.dma_start(taps["ot"].ap(), ot_sb[:])
        nc.sync.dma_start(
            taps["ocat"].ap().rearrange("(c p) l -> p c l", p=P), ocat_sb[:]
        )
        nc.sync.dma_start(taps["qk"].ap(), qk_sb[:])
        nc.sync.dma_start(taps["vaug"].ap(), vaug_sb[:])

    # ---- output projection: y[lq] = Ocat^T.T @ wout (full L, redundant) ----
    for lq in range(L // P):
        ps = psum.tile([P, 1024], F32, tag="sc", bufs=2)
        for nhf in range(E // 512):
            for e in range(EC):
                nc.tensor.matmul(
                    ps[:, nhf * 512 : (nhf + 1) * 512],
                    lhsT=ocat_sb[:, e, lq * P : (lq + 1) * P],
                    rhs=wout_sb[:, e, nhf * 512 : (nhf + 1) * 512],
                    start=(e == 0),
                    stop=(e == EC - 1),
                )
        yt = work.tile([P, E], F32, tag="yt", bufs=2)
        nc.scalar.copy(out=yt[:], in_=ps[:, :E])
        nc.sync.dma_start(y.ap()[lq * P : (lq + 1) * P, :], yt[:])

    ctx.close()


@functools.lru_cache(maxsize=2)
def build_module(L=L_FULL, E=E_FULL, HC=H_FULL // 2, D=64, asserts=False,
                 debug_taps=False, use_collective=True):
    nc = bacc.Bacc(
        "TRN2",
        target_bir_lowering=False,
        debug=False,
        enable_asserts=asserts,
        num_devices=N_CORES,
    )
    A = HC * D
    xT = nc.dram_tensor("xT", [E, L], BF16, kind="ExternalInput")
    wqkv = nc.dram_tensor("wqkv", [E, 3 * A], BF16, kind="ExternalInput")
    wout = nc.dram_tensor("wout", [E, E], BF16, kind="ExternalInput")
    cosT = nc.dram_tensor("cosT", [128, L], BF16, kind="ExternalInput")
    sinT = nc.dram_tensor("sinT", [128, L], BF16, kind="ExternalInput")
    y = nc.dram_tensor("y", [L, E], F32, kind="ExternalOutput")
    taps = None
    if debug_taps:
        taps = {
            "ot": nc.dram_tensor("ot_dbg", [64, HC, L], BF16, kind="ExternalOutput"),
            "ocat": nc.dram_tensor("ocat_dbg", [E, L], BF16, kind="ExternalOutput"),
            "qk": nc.dram_tensor("qk_dbg", [128, 2, HC // 2, L], BF16, kind="ExternalOutput"),
            "vaug": nc.dram_tensor(
                "vaug_dbg", [128, L // 128, HC, D + 1], BF16, kind="ExternalOutput"
            ),
        }
    with tile.TileContext(nc) as tc:
        _emit(tc, nc, xT, wqkv, wout, cosT, sinT, y, L, E, HC, D, taps=taps,
              use_collective=use_collective)
    nc.compile()
    return nc


def _rope_tables(L, D):
    """cos/sin tables in the de-interleaved 32-row layout, stacked x4.

    Row p (p in [0,32)): frequency p (covers original dims 2p / 2p+1).
    sin is pre-signed for the post-swap add: blocks [+s, -s, +s, -s].
    """
    half = D // 2
    inv_freq = 1.0 / (ROPE_THETA ** (np.arange(0, D, 2, dtype=np.float64) / D))
    freqs = np.arange(L, dtype=np.float64)[None, :] * inv_freq[:, None]  # [32, L]
    cos32 = np.cos(freqs)
    sin32 = np.sin(freqs)
    bf = ml_dtypes.bfloat16
    cos = np.tile(cos32, (128 // half, 1)).astype(bf)
    sin_block = np.concatenate([sin32, -sin32], axis=0)  # [64, L]
    sin = np.tile(sin_block, (2, 1)).astype(bf)
    return cos, sin


def _deint_cols(base, h, D):
    """Column indices of head h (offset base), even dims then odd dims."""
    cols = base + h * D + np.arange(D)
    return np.concatenate([cols[0::2], cols[1::2]])


def make_core_inputs(x, w_qkv, w_out, H=H_FULL, D=64):
    """Per-core input dicts from the full (unsharded) fp32 inputs."""
    Bv, L, E = x.shape
    HC = H // (N_CORES // Bv)
    A_full = H * D
    bf = ml_dtypes.bfloat16
    cos, sin = _rope_tables(L, D)
    wout_bf = np.ascontiguousarray(w_out).astype(bf)
    in_maps = []
    for c in range(N_CORES):
        b, g = c // 2, c % 2
        xT = np.ascontiguousarray(x[b].T).astype(bf)
        qcols = []
        kcols = []
        vcols = []
        for p in range(HC // 2):
            for hh in range(2):
                h = g * HC + 2 * p + hh
                qcols.append(_deint_cols(0, h, D))
                kcols.append(_deint_cols(A_full, h, D))
        for hl in range(HC):
            h = g * HC + hl
            vcols.append(2 * A_full + h * D + np.arange(D))
        cols = np.concatenate(qcols + kcols + vcols)
        wqkv_c = np.ascontiguousarray(w_qkv[:, cols]).astype(bf)
        in_maps.append(
            {
                "xT": xT,
                "wqkv": wqkv_c,
                "wout": wout_bf,
                "cosT": cos[:, :L].copy(),
                "sinT": sin[:, :L].copy(),
            }
        )
    return in_maps


def kernel(x, w_qkv, w_out):
    x = np.asarray(x)
    w_qkv = np.asarray(w_qkv)
    w_out = np.asarray(w_out)
    Bv, L, E = x.shape
    nc = build_module(L=L, E=E)
    in_maps = make_core_inputs(x, w_qkv, w_out)
    res = run_bass_kernel_spmd(nc, in_maps, core_ids=list(range(N_CORES)))
    out = np.empty((Bv, L, E), dtype=np.float32)
    for b in range(Bv):
        out[b] = res.results[2 * b]["y"]
    return out



# revision 14
# speedup vs baseline: 1.0652x; 1.0652x over previous
"""Multi-head self-attention with RoPE — Trainium2 Bass kernel, 8 NeuronCores.

Sharding: core c = 2*b + g handles batch b = c//2 and head-group g = c%2
(8 of the 16 heads).  Within each batch pair the cores AllGather their
normalized attention outputs (O^T, bf16) and both run the full output
projection redundantly; the host keeps the even core's y.  No reduction
on the host.

Per-core dataflow (matmuls bf16, fp32 PSUM accumulation):
  xT [E, L] bf16 (pre-transposed on host)
  QKV:   Q^T/K^T pair tiles via W-stationary matmuls; V natural [L, 512].
  RoPE:  weights pre-permuted on host to de-interleave even/odd dims, so
         rotate-half becomes a 32-partition block swap (SBUF->SBUF DMA);
         cos/sin multiplies on GPSIMD, combine add on DVE.
  Scores:S^T half-tiles [Lk=128, Lq=512] per head, heads row-tiled on PE;
         three halves share a [128, 1536] PSUM tile (double buffered) so
         exp runs as few, wide ACT instructions overlapped with PE.
  Softmax: denominator via ones-column appended to V (PSUM partition 64
         of O^T); reciprocal_approx + gpsimd partition_broadcast.
  AV:    O^T[65, 512] += V_aug^T A^T over 16 Lk chunks.
  Proj:  y[lq] = Ocat^T.T @ w_out, fp32 [L, E].
"""

import contextlib
import functools

import numpy as np
import ml_dtypes

import concourse.bass as bass
import concourse.mybir as mybir
import concourse.tile as tile
from concourse import bacc
from concourse.bass_utils import run_bass_kernel_spmd

BF16 = mybir.dt.bfloat16
F32 = mybir.dt.float32
N_CORES = 8
ROPE_THETA = 10000.0

B_FULL, L_FULL, E_FULL = 4, 2048, 1024
H_FULL = 16


def _emit(tc, nc, xT, wqkv, wout, cosT, sinT, y, L, E, HC, D, taps=None, use_collective=True):
    P = 128
    EC = E // P                 # E chunks of 128 (contraction)
    NPAIR = HC // 2             # head pairs per core
    LT = L // 512               # 512-wide L tiles
    LKC = L // P                # 128-wide Lk chunks
    A = HC * D                  # local attention width (512)
    scale = 1.0 / float(np.sqrt(D))
    Exp = mybir.ActivationFunctionType.Exp

    ctx = contextlib.ExitStack()
    pool = ctx.enter_context(tc.tile_pool(name="sb", bufs=1))
    psum = ctx.enter_context(tc.tile_pool(name="ps", bufs=1, space="PSUM"))
    work = ctx.enter_context(tc.tile_pool(name="wk", bufs=1))
    dram = ctx.enter_context(tc.tile_pool(name="dr", bufs=1, space="DRAM"))

    # ---- persistent SBUF buffers ----
    xt_sb = pool.tile([P, EC, L], BF16, tag="xbuf")
    wqkv_sb = pool.tile([P, EC, 3 * A], BF16, tag="wqkv")
    wout_sb = pool.tile([P, EC, E // 2], BF16, tag="wout")
    cos_sb = pool.tile([P, L], BF16, tag="costab")
    sin_sb = pool.tile([P, L], BF16, tag="sintab")
    qk_sb = pool.tile([P, 2, NPAIR, L], BF16, tag="qk")      # [pair-rows, q/k, pair, L]
    vaug_sb = pool.tile([P, LKC, HC, D + 1], BF16, tag="vaug")
    ot_sb = pool.tile([64, HC, L], BF16, tag="ot")           # normalized O^T per head

    nc.sync.dma_start(xt_sb[:], xT.ap().rearrange("(c p) l -> p c l", p=P))
    nc.sync.dma_start(wqkv_sb[:], wqkv.ap().rearrange("(c p) n -> p c n", p=P))
    nc.sync.dma_start(wout_sb[:], wout.ap().rearrange("(c p) n -> p c n", p=P))
    nc.sync.dma_start(cos_sb[:], cosT.ap())
    nc.sync.dma_start(sin_sb[:], sinT.ap())

    # ones column for the softmax denominator
    nc.vector.memset(vaug_sb[:, :, :, D : D + 1], 1.0)

    # ---- V = x @ Wv, natural [L, A] layout, 2 L-chunks per PSUM tile ----
    for vg in range(LKC // 2):
        ps = psum.tile([P, 1024], F32, tag="sc", bufs=2)
        for i in range(2):
            lt = vg * 2 + i
            for e in range(EC):
                nc.tensor.matmul(
                    ps[:, i * 512 : (i + 1) * 512],
                    lhsT=xt_sb[:, e, lt * P : (lt + 1) * P],
                    rhs=wqkv_sb[:, e, 2 * A : 3 * A],
                    start=(e == 0),
                    stop=(e == EC - 1),
                )
        nc.scalar.copy(
            out=vaug_sb[:, vg * 2 : (vg + 1) * 2, :, 0:D],
            in_=ps[:].rearrange("p (t h d) -> p t h d", h=HC, d=D),
        )

    # ---- Q^T / K^T + RoPE ----
    # psum tile cols: [q | k] for one 512-wide L tile
    for p in range(NPAIR):
        for lt in range(LT):
            ps = psum.tile([P, 1024], F32, tag="sc", bufs=2)
            for qk in range(2):
                wcol = qk * A + p * P
                for e in range(EC):
                    nc.tensor.matmul(
                        ps[:, qk * 512 : (qk + 1) * 512],
                        lhsT=wqkv_sb[:, e, wcol : wcol + P],
                        rhs=xt_sb[:, e, lt * 512 : (lt + 1) * 512],
                        start=(e == 0),
                        stop=(e == EC - 1),
                    )
            Lsl = slice(lt * 512, (lt + 1) * 512)
            tab = lambda sb: (
                sb[:, Lsl][:, None, :].to_broadcast([P, 2, 512])
            )
            qs = work.tile([P, 1024], BF16, tag="qs", bufs=3)
            nc.scalar.copy(out=qs[:], in_=ps[:])
            qs_v = qs[:].rearrange("p (q c) -> p q c", q=2)
            w = work.tile([P, 1024], BF16, tag="w", bufs=3)
            t = work.tile([P, 1024], BF16, tag="w", bufs=3)
            nc.vector.tensor_mul(w[:].rearrange("p (q c) -> p q c", q=2), qs_v, tab(sin_sb))
            nc.vector.tensor_mul(t[:].rearrange("p (q c) -> p q c", q=2), qs_v, tab(cos_sb))
            wsw = work.tile([P, 1024], BF16, tag="wsw", bufs=2)
            for blk in range(4):
                sb = blk ^ 1  # swap 32-row blocks pairwise
                nc.sync.dma_start(
                    wsw[blk * 32 : (blk + 1) * 32, :], w[sb * 32 : (sb + 1) * 32, :]
                )
            out_ap = qk_sb[:, :, p, Lsl]  # [P, 2, 512]
            nc.vector.tensor_add(
                out_ap,
                t[:].rearrange("p (q c) -> p q c", q=2),
                wsw[:].rearrange("p (q c) -> p q c", q=2),
            )

    # ---- attention + AllGather staging (one collective per pair-of-pairs) ----
    cc_half = 2 * P  # feature rows per collective (2 pairs x 128)
    cc_in = [
        dram.tile([cc_half, L], BF16, tag=f"ccin{i}", bufs=1, name=f"ccin{i}")
        for i in range(2)
    ]
    cc_out = [
        dram.tile([2, cc_half, L], BF16, tag=f"ccout{i}", bufs=1, name=f"ccout{i}")
        for i in range(2)
    ]
    # reuses xt's SBUF slot (xT is consumed by the end of the QKV phase)
    ocat_sb = pool.tile([P, EC, L], BF16, tag="xbuf")

    for p in range(NPAIR):
        h0, h1 = 2 * p, 2 * p + 1
        for lq in range(LT):
            ot0 = psum.tile([65, 512], F32, tag="ot", bufs=2)
            ot1 = psum.tile([65, 512], F32, tag="ot", bufs=2)
            ots = (ot0, ot1)
            Lq = slice(lq * 512, (lq + 1) * 512)
            # halves: (head, lk) pairs in lk-major order, grouped 3 per
            # [128, 1536] psum tile so exp runs as wide ACT instructions.
            halves = [(hh, lk) for lk in range(LKC) for hh in range(2)]
            gi = 0
            while gi < len(halves):
                grp = halves[gi : gi + 3]
                nh = len(grp)
                ps = psum.tile([P, 1536], F32, tag="sc", bufs=2)
                for j, (hh, lk) in enumerate(grp):
                    nc.tensor.matmul(
                        ps[:, j * 512 : (j + 1) * 512],
                        lhsT=qk_sb[hh * 64 : (hh + 1) * 64, 1, p, lk * P : (lk + 1) * P],
                        rhs=qk_sb[hh * 64 : (hh + 1) * 64, 0, p, Lq],
                        start=True,
                        stop=True,
                    )
                at = work.tile([P, 1536], BF16, tag="at", bufs=4)
                nc.scalar.activation(at[:, : nh * 512], ps[:, : nh * 512], Exp, scale=scale)
                for j, (hh, lk) in enumerate(grp):
                    nc.tensor.matmul(
                        ots[hh][:],
                        lhsT=vaug_sb[:, lk, 2 * p + hh, :],
                        rhs=at[:, j * 512 : (j + 1) * 512],
                        start=(lk == 0),
                        stop=(lk == LKC - 1),
                    )
                gi += nh
            for hh, otp in ((0, ot0), (1, ot1)):
                # denominator: PSUM row 64 -> SBUF row 64 -> (DMA) row 0 ->
                # reciprocal -> broadcast to 64 partitions -> scale O^T.
                # (the DMA hop exists because DVE lanes are partition-locked)
                den = work.tile([65, 512], F32, tag="den", bufs=2)
                nc.vector.tensor_copy(out=den[64:65, :], in_=otp[64:65, :])
                den0 = work.tile([1, 512], F32, tag="den0", bufs=2)
                nc.sync.dma_start(den0[0:1, :], den[64:65, :])
                rec0 = work.tile([1, 512], F32, tag="rec0", bufs=2)
                nc.vector.reciprocal_approx_fast(rec0[0:1, :], den0[0:1, :])
                rbc = work.tile([64, 512], F32, tag="rbc", bufs=2)
                nc.gpsimd.partition_broadcast(rbc[:], rec0[0:1, :])
                nc.vector.tensor_mul(ot_sb[:, 2 * p + hh, Lq], otp[0:64, :], rbc[:])
        if p % 2 == 1:
            half = p // 2
            src = ot_sb[:, half * 4 : half * 4 + 4, :]
            nc.sync.dma_start(
                cc_in[half][:].rearrange("(h d) l -> d h l", d=64),
                src,
            )
            if use_collective:
                nc.gpsimd.collective_compute(
                    "AllGather",
                    mybir.AluOpType.bypass,
                    replica_groups=[[2 * i, 2 * i + 1] for i in range(N_CORES // 2)],
                    ins=[cc_in[half][:].opt()],
                    outs=[cc_out[half][:].opt()],
                )
            else:  # timing-analysis build: stand-in DMAs, no collective
                nc.sync.dma_start(cc_out[half][0], cc_in[half][:])
                nc.sync.dma_start(cc_out[half][1], cc_in[half][:])
            # gather both cores' copies into Ocat^T feature chunks right away
            for g2 in range(2):
                for q in range(2):
                    nc.sync.dma_start(
                        ocat_sb[:, g2 * NPAIR + half * 2 + q, :],
                        cc_out[half][g2, q * P : (q + 1) * P, :],
                    )

    if taps is not None:
        nc.sync.dma_start(taps["ot"].ap(), ot_sb[:])
        nc.sync.dma_start(
            taps["ocat"].ap().rearrange("(c p) l -> p c l", p=P), ocat_sb[:]
        )
        nc.sync.dma_start(taps["qk"].ap(), qk_sb[:])
        nc.sync.dma_start(taps["vaug"].ap(), vaug_sb[:])

    # ---- output projection: y[lq] = Ocat^T.T @ wout_half ([L, 512] per
    # core; the host stitches the two column halves) ----
    for lq in range(L // P):
        ps = psum.tile([P, 512], F32, tag="sc", bufs=2)
        for e in range(EC):
            nc.tensor.matmul(
                ps[:],
                lhsT=ocat_sb[:, e, lq * P : (lq + 1) * P],
                rhs=wout_sb[:, e, :],
                start=(e == 0),
                stop=(e == EC - 1),
            )
        yt = work.tile([P, 512], F32, tag="yt", bufs=2)
        nc.vector.tensor_copy(out=yt[:], in_=ps[:])
        nc.sync.dma_start(y.ap()[lq * P : (lq + 1) * P, :], yt[:])

    ctx.close()


@functools.lru_cache(maxsize=2)
def build_module(L=L_FULL, E=E_FULL, HC=H_FULL // 2, D=64, asserts=False,
                 debug_taps=False, use_collective=True):
    nc = bacc.Bacc(
        "TRN2",
        target_bir_lowering=False,
        debug=False,
        enable_asserts=asserts,
        num_devices=N_CORES,
    )
    A = HC * D
    xT = nc.dram_tensor("xT", [E, L], BF16, kind="ExternalInput")
    wqkv = nc.dram_tensor("wqkv", [E, 3 * A], BF16, kind="ExternalInput")
    wout = nc.dram_tensor("wout", [E, E // 2], BF16, kind="ExternalInput")
    cosT = nc.dram_tensor("cosT", [128, L], BF16, kind="ExternalInput")
    sinT = nc.dram_tensor("sinT", [128, L], BF16, kind="ExternalInput")
    y = nc.dram_tensor("y", [L, E // 2], F32, kind="ExternalOutput")
    taps = None
    if debug_taps:
        taps = {
            "ot": nc.dram_tensor("ot_dbg", [64, HC, L], BF16, kind="ExternalOutput"),
            "ocat": nc.dram_tensor("ocat_dbg", [E, L], BF16, kind="ExternalOutput"),
            "qk": nc.dram_tensor("qk_dbg", [128, 2, HC // 2, L], BF16, kind="ExternalOutput"),
            "vaug": nc.dram_tensor(
                "vaug_dbg", [128, L // 128, HC, D + 1], BF16, kind="ExternalOutput"
            ),
        }
    with tile.TileContext(nc) as tc:
        _emit(tc, nc, xT, wqkv, wout, cosT, sinT, y, L, E, HC, D, taps=taps,
              use_collective=use_collective)
    nc.compile()
    return nc


def _rope_tables(L, D):
    """cos/sin tables in the de-interleaved 32-row layout, stacked x4.

    Row p (p in [0,32)): frequency p (covers original dims 2p / 2p+1).
    sin is pre-signed for the post-swap add: blocks [+s, -s, +s, -s].
    """
    half = D // 2
    inv_freq = 1.0 / (ROPE_THETA ** (np.arange(0, D, 2, dtype=np.float64) / D))
    freqs = np.arange(L, dtype=np.float64)[None, :] * inv_freq[:, None]  # [32, L]
    cos32 = np.cos(freqs)
    sin32 = np.sin(freqs)
    bf = ml_dtypes.bfloat16
    cos = np.tile(cos32, (128 // half, 1)).astype(bf)
    sin_block = np.concatenate([sin32, -sin32], axis=0)  # [64, L]
    sin = np.tile(sin_block, (2, 1)).astype(bf)
    return cos, sin


def _deint_cols(base, h, D):
    """Column indices of head h (offset base), even dims then odd dims."""
    cols = base + h * D + np.arange(D)
    return np.concatenate([cols[0::2], cols[1::2]])


def make_core_inputs(x, w_qkv, w_out, H=H_FULL, D=64):
    """Per-core input dicts from the full (unsharded) fp32 inputs."""
    Bv, L, E = x.shape
    HC = H // (N_CORES // Bv)
    A_full = H * D
    bf = ml_dtypes.bfloat16
    cos, sin = _rope_tables(L, D)
    E_out = w_out.shape[1]
    wout_halves = [
        np.ascontiguousarray(w_out[:, g * (E_out // 2) : (g + 1) * (E_out // 2)]).astype(bf)
        for g in range(2)
    ]
    in_maps = []
    for c in range(N_CORES):
        b, g = c // 2, c % 2
        xT = np.ascontiguousarray(x[b].T).astype(bf)
        qcols = []
        kcols = []
        vcols = []
        for p in range(HC // 2):
            for hh in range(2):
                h = g * HC + 2 * p + hh
                qcols.append(_deint_cols(0, h, D))
                kcols.append(_deint_cols(A_full, h, D))
        for hl in range(HC):
            h = g * HC + hl
            vcols.append(2 * A_full + h * D + np.arange(D))
        cols = np.concatenate(qcols + kcols + vcols)
        wqkv_c = np.ascontiguousarray(w_qkv[:, cols]).astype(bf)
        in_maps.append(
            {
                "xT": xT,
                "wqkv": wqkv_c,
                "wout": wout_halves[g],
                "cosT": cos[:, :L].copy(),
                "sinT": sin[:, :L].copy(),
            }
        )
    return in_maps


def kernel(x, w_qkv, w_out):
    x = np.asarray(x)
    w_qkv = np.asarray(w_qkv)
    w_out = np.asarray(w_out)
    Bv, L, E = x.shape
    nc = build_module(L=L, E=E)
    in_maps = make_core_inputs(x, w_qkv, w_out)
    res = run_bass_kernel_spmd(nc, in_maps, core_ids=list(range(N_CORES)))
    out = np.empty((Bv, L, E), dtype=np.float32)
    for b in range(Bv):
        out[b, :, : E // 2] = res.results[2 * b]["y"]
        out[b, :, E // 2 :] = res.results[2 * b + 1]["y"]
    return out

